# revision 28
# baseline (speedup 1.0000x reference)
"""3-layer GCN (GCNConv+BN+ReLU x2, GCNConv, softmax) on 8 Trainium2 NeuronCores.

Strategy (graph/data parallel, per sharding hint):
  - Nodes sharded 6250/core. Edges partitioned by destination core, sorted by
    dst, grouped into 128-dst windows, padded to 128-edge tiles.
  - Symmetric normalization coef = dinv[src]*dinv[dst] is factored into
    per-node pre/post scaling (dinv; the input-side factor is folded into x
    on the host), so per-edge work is a pure row gather + one-hot matmul.
  - BatchNorm (eval) folded into the conv weights/bias on the host. x is
    shipped pre-transposed [IN, NS] so stage A feeds matmul lhsT directly.
  - H1/H2 stored fp8e4 (256B rows = min gather elem; halves AG wire + gather
    bytes; rel err stays ~2.6e-4 vs the 2e-2 gate). H3 stays bf16x128.
  - Per layer: H_local = act_local @ W (TensorE), AllGather H (on one chip
    the collective is nearly free), then per dst-window: dma_gather rows of
    H, build one-hot P via iota is_equal (VectorE), accumulate P.T @ msg
    into PSUM (TensorE), postprocess (bias/relu/dinv scaling).
  - SELF-LOOPS bypass the gather stream entirely (noloop=True): each
    window's self contribution is its own contiguous h_loc rows, added via
    one identity matmul from an HWDGE dma_start — zero SWDGE descriptors.
  - Node permutation (balance=True): nodes are pair-dealt (pairs matched by
    opposite lo/hi indegree skew so every pair is bucket-neutral, snaked by
    total weight) + swap-repaired so per-(window,bucket) indegree maxima
    over cores sit under the 128-row tile boundary. With self-loops removed
    this cuts the padded tile count NTT ~967 -> ~797 (-18% descriptors).
  - BLOCK-SPLIT HALO EXCHANGE: each layer's H is stored as two row-blocks
    (windows 0..23 of every core = block-lo [8x3072], windows 24..48 =
    block-hi [8x3178], both int16-addressable) that AllGather
    independently. AG-lo fires right after window 23's conv — mid-stage —
    and AG-hi at stage end, so AG wire hides under gather streams.
    Aggregation is two-pass: pass 0 consumes the lo-block gather stream (+
    self rows) into per-window SBUF f32 accumulators while AG-hi is still
    on the wire; pass 1 consumes the hi stream, combines, postprocesses,
    and feeds the next conv. Each pass spreads its single stream across
    ALL 4 SWDGE queues.
  - Gathers use int16 indices into the block tables, fetched in 8-tile
    chunks (1024 indices/call — the HW limit for one SWDGE gather; 2048
    wedges the device).
  - PERF-CRITICAL: gathers are SWDGE-drain bound (~2.9us/1024x256B call at 4
    queues = per-queue DMA engine bandwidth); num_swdge_queues=4 is the
    ucode max; dynamic_dma_scratch_size=65536 keeps the per-queue descriptor
    rings from serializing desc-gen against drain. The gather streams are
    ~85% of the serialized per-rep makespan.
  - Final layer: width-64 aggregation + softmax, output f32 shards.
Measured (serialized rep slope, inter-rep barrier): baseline 1.67 ->
1.20 (noloop+balance+xsT, single-pass) -> ~0.95-1.02ms/rep (block-split,
gbufs=12).
"""
import sys
import time

sys.path.insert(0, "/opt/trn_rl_repo")

import numpy as np
import ml_dtypes

from concourse import bass, mybir, bacc, tile
from concourse import bass_utils

# problem constants (hardcoded per contract)
N, E = 50000, 800000
IN = 256
HID = 256
OUT = 64
OUTP = 128  # padded width for layer-3 H (dma_gather needs >=256B rows)
C = 8
NS = N // C  # 6250 nodes per core
P = 128
NW = (NS + P - 1) // P  # 49 windows per core
SPLIT = 25000  # src-index split so indices fit int16
EPS = 1e-5
CHUNK = 8  # tiles per dma_gather call (8*128 = 1024 idx = HW limit)
# block split: local rows [0, GSPLIT) = windows 0..23 form block-lo across all
# cores (8*3072 = 24576 rows, int16-addressable); rows [GSPLIT, NS) = windows
# 24..48 form block-hi (8*3178 = 25424 rows). Each block AllGathers
# independently, so AG-lo fires after window 23 and AG wire hides under the
# gather streams.
GSPLIT = 24 * P  # 3072
NWLO = 24
BL0 = C * GSPLIT  # 24576
BL1 = C * (NS - GSPLIT)  # 25424

BF = mybir.dt.bfloat16
F8 = mybir.dt.float8e4
F32 = mybir.dt.float32
I16 = mybir.dt.int16

AX = mybir.AluOpType
AF = mybir.ActivationFunctionType


# --------------------------------------------------------------------------
# host-side preprocessing
# --------------------------------------------------------------------------
def _balance_perm(a_in, b_in):
    """Permute nodes (within src row-blocks) so per-(window,bucket) in-degree
    sums are balanced across cores: pair nodes with opposite (a-b) skew so
    each pair is bucket-neutral, snake-deal pairs by total weight over the
    block's windows, then swap-repair windows whose max-over-core sum sits
    just above a 128-row tile boundary. perm[new_id] = old_id."""
    perm = np.empty(N, np.int64)
    allpos = np.arange(N)
    groups = [
        (allpos[(allpos % NS) < GSPLIT], range(0, NWLO)),
        (allpos[(allpos % NS) >= GSPLIT], range(NWLO, NW)),
    ]
    for nodes, wrange in groups:
        o = nodes[np.argsort(a_in[nodes] - b_in[nodes], kind="stable")]
        half = len(o) // 2
        first, second = o[:half], o[half:][::-1]
        pw = a_in[first] + b_in[first] + a_in[second] + b_in[second]
        po = np.argsort(-pw, kind="stable")
        first, second = first[po], second[po]
        wins = []
        for c in range(C):
            for wl_ in wrange:
                base = c * NS + wl_ * P
                wins.append((base, min(P, NS - wl_ * P)))
        bases = np.array([b for b, _ in wins])
        caps = np.array([cp for _, cp in wins])
        fill = np.zeros(len(wins), np.int64)
        pos, r = 0, 0
        while pos < half:
            active = np.where(fill + 2 <= caps)[0]
            if len(active) == 0:
                break
            if r % 2 == 1:
                active = active[::-1]
            take = min(len(active), half - pos)
            sel = active[:take]
            perm[bases[sel] + fill[sel]] = first[pos : pos + take]
            perm[bases[sel] + fill[sel] + 1] = second[pos : pos + take]
            fill[sel] += 2
            pos += take
            r += 1
        rem = list(first[pos:]) + list(second[pos:])
        ri = 0
        for j in np.where(fill < caps)[0]:
            while fill[j] < caps[j]:
                perm[bases[j] + fill[j]] = rem[ri]
                ri += 1
                fill[j] += 1
        assert ri == len(rem)
    return perm


def _repair_perm(perm, a_in, b_in, iters=24):
    """Greedy swap-repair on the permuted layout: for (window,bucket) cells
    whose max-over-core sum is just over a tile boundary, swap a heavy node
    out (within the same core) for a light one from a window with headroom."""
    aw = np.stack([a_in, b_in], 1)  # [N,2]

    def cells(perm):
        # cnt[c,w,b] = sum of bucket-b indegree of nodes in (c,w)
        cnt = np.zeros((C, NW, 2), np.int64)
        for c in range(C):
            for w in range(NW):
                sl = perm[c * NS + w * P : c * NS + w * P + min(P, NS - w * P)]
                cnt[c, w, 0] = a_in[sl].sum()
                cnt[c, w, 1] = b_in[sl].sum()
        return cnt

    cnt = cells(perm)
    for _ in range(iters):
        K = np.ceil(cnt.max(axis=0) / P).astype(np.int64)  # [NW,2]
        ntt0 = int(K.sum())
        # candidate cells: overflow above (K-1)*P boundary, smallest first
        cand = []
        for w in range(NW):
            for b in range(2):
                lim = (K[w, b] - 1) * P
                gap = int(cnt[:, w, b].max() - lim)
                if 0 < gap <= 64:
                    cand.append((gap, w, b))
        if not cand:
            break
        cand.sort()
        moved = 0
        for gap, w, b in cand[:40]:
            lim = (K[w, b] - 1) * P
            for c in np.where(cnt[:, w, b] > lim)[0]:
                sl_lo = c * NS + w * P
                nrow = min(P, NS - w * P)
                sl = perm[sl_lo : sl_lo + nrow]
                # try donor windows on same core AND same row-block (swaps
                # must not move nodes across the block boundary) with
                # headroom in BOTH buckets
                for w2 in np.argsort(cnt[c, :, b]):
                    if w2 == w or (w2 < NWLO) != (w < NWLO):
                        continue
                    need = int(cnt[c, w, b] - lim)
                    if need <= 0:
                        break
                    s2_lo = c * NS + w2 * P
                    nrow2 = min(P, NS - w2 * P)
                    sl2 = perm[s2_lo : s2_lo + nrow2]
                    d1 = aw[sl, b]
                    d2 = aw[sl2, b]
                    u_i = int(np.argmax(d1))
                    v_i = int(np.argmin(d2))
                    delta = int(d1[u_i] - d2[v_i])
                    if delta <= 0:
                        continue
                    # other-bucket delta
                    ob = 1 - b
                    delta_o = int(aw[sl[u_i], ob] - aw[sl2[v_i], ob])
                    # check donor window stays under ITS boundaries
                    if cnt[c, w2, b] + delta > K[w2, b] * P:
                        continue
                    if cnt[c, w2, ob] + (-delta_o) > K[w2, ob] * P:
                        continue
                    if cnt[c, w, ob] + delta_o > K[w, ob] * P:
                        continue
                    u, v = sl[u_i], sl2[v_i]
                    perm[sl_lo + u_i], perm[s2_lo + v_i] = v, u
                    cnt[c, w, b] -= delta
                    cnt[c, w2, b] += delta
                    cnt[c, w, ob] -= delta_o
                    cnt[c, w2, ob] += delta_o
                    sl = perm[sl_lo : sl_lo + nrow]
                    moved += 1
        Knew = np.ceil(cnt.max(axis=0) / P).astype(np.int64)
        if int(Knew.sum()) >= ntt0 and moved == 0:
            break
    return perm


def _host_prep(inputs, chunk=CHUNK, balance=True, noloop=True, srcsort=False,
               pres=False):
    x = np.asarray(inputs["x"], np.float32)
    ei = np.asarray(inputs["edge_index"]).astype(np.int64)
    W1 = np.asarray(inputs["W1"], np.float32)
    b1 = np.asarray(inputs["b1"], np.float32)
    g1 = np.asarray(inputs["g1"], np.float32)
    beta1 = np.asarray(inputs["beta1"], np.float32)
    m1 = np.asarray(inputs["m1"], np.float32)
    v1 = np.asarray(inputs["v1"], np.float32)
    W2 = np.asarray(inputs["W2"], np.float32)
    b2 = np.asarray(inputs["b2"], np.float32)
    g2 = np.asarray(inputs["g2"], np.float32)
    beta2 = np.asarray(inputs["beta2"], np.float32)
    m2 = np.asarray(inputs["m2"], np.float32)
    v2 = np.asarray(inputs["v2"], np.float32)
    W3 = np.asarray(inputs["W3"], np.float32)
    b3 = np.asarray(inputs["b3"], np.float32)

    # gather-stream edge set: self-loops handled separately (noloop) via a
    # contiguous per-window identity add from h_loc — no gather descriptors.
    loops = np.arange(N, dtype=np.int64)
    if noloop:
        src = ei[0].copy()
        dst = ei[1].copy()
    else:
        src = np.concatenate([ei[0], loops])
        dst = np.concatenate([ei[1], loops])

    # Node permutation, constrained to permute within each src row-block (so
    # each node's lo/hi source-bucket is preserved): balances each window's
    # (lo-indegree, hi-indegree) jointly, shrinking the max-over-cores
    # ceil(cnt/128) tile padding. perm[new_id] = old_id.
    a_in = np.bincount(dst[(src % NS) < GSPLIT], minlength=N).astype(np.int64)
    b_in = np.bincount(dst[(src % NS) >= GSPLIT], minlength=N).astype(np.int64)

    if balance:
        perm = _balance_perm(a_in, b_in)
        perm = _repair_perm(perm, a_in, b_in)
    else:
        perm = np.arange(N, dtype=np.int64)
    inv = np.empty(N, np.int64)
    inv[perm] = np.arange(N)

    x = x[perm]
    src = inv[src]
    dst = inv[dst]

    # degree INCLUDES the self-loop (reference semantics) regardless of noloop
    deg = np.bincount(dst, minlength=N).astype(np.float32)
    if noloop:
        deg = deg + 1.0
    dinv = (1.0 / np.sqrt(np.maximum(deg, 1.0))).astype(np.float32)

    # fold BN into conv weights/bias
    s1 = g1 / np.sqrt(v1 + EPS)
    W1e = W1 * s1[None, :]
    c1 = (b1 - m1) * s1 + beta1
    s2 = g2 / np.sqrt(v2 + EPS)
    W2e = W2 * s2[None, :]
    c2 = (b2 - m2) * s2 + beta2
    W3e = np.concatenate([W3, np.zeros((HID, OUTP - OUT), np.float32)], axis=1)
    c3 = b3

    # edge grouping: (owner core, window, lo/hi src row-block)
    owner = dst // NS
    wl = (dst % NS) // P
    bkt = ((src % NS) >= GSPLIT).astype(np.int64)
    grp = (owner * NW + wl) * 2 + bkt
    # NOTE: sorting by src within groups (lexsort((src, grp))) was measured
    # SLOWER on HW — correlated ascending streams from all 8 cores contend on
    # the same HBM banks. Keep arrival order.
    order = np.lexsort((src, grp)) if srcsort else np.argsort(grp, kind="stable")
    g = grp[order]
    ss = src[order]
    dd = dst[order]

    cnt = np.bincount(g, minlength=C * NW * 2)
    cnt3 = cnt.reshape(C, NW, 2)
    KA = np.ceil(cnt3[:, :, 0].max(axis=0) / P).astype(np.int64)  # [NW]
    KB = np.ceil(cnt3[:, :, 1].max(axis=0) / P).astype(np.int64)
    K = KA + KB
    # window-contiguous tile order (for loc / P / matmuls)
    tile_base = np.concatenate([[0], np.cumsum(K)]).astype(np.int64)
    NTT = int(K.sum())
    # bucket stream order (for gather chunks)
    lo_base = np.concatenate([[0], np.cumsum(KA)]).astype(np.int64)
    hi_base = np.concatenate([[0], np.cumsum(KB)]).astype(np.int64)
    NLO = int(KA.sum())
    NHI = int(KB.sum())
    NLOp = (NLO + chunk - 1) // chunk * chunk
    NHIp = (NHI + chunk - 1) // chunk * chunk

    starts = np.concatenate([[0], np.cumsum(cnt)]).astype(np.int64)
    rank = np.arange(g.size, dtype=np.int64) - starts[g]
    ow = g >> 1
    c_of = ow // NW
    w_of = ow % NW
    b_of = g & 1
    # stream slot (gather order): position within lo/hi stream
    sslot = np.where(
        b_of == 0, lo_base[w_of] + rank // P, hi_base[w_of] + rank // P
    )
    # window slot (matmul/P order)
    wslot = tile_base[w_of] + np.where(b_of == 0, 0, KA[w_of]) + rank // P
    part = rank % P

    # source position within its block table: block-lo rows are laid out
    # core-major [core*GSPLIT + r], block-hi [core*(NS-GSPLIT) + (r-GSPLIT)]
    s_core = ss // NS
    s_r = ss % NS
    pos_lo = s_core * GSPLIT + s_r
    pos_hi = s_core * (NS - GSPLIT) + (s_r - GSPLIT)

    idx16 = np.zeros((C, P, 8 * (NLOp + NHIp)), np.int16)
    locv = np.full((C, P, NTT, 1), -1.0, np.float32)
    for c in range(C):
        m = c_of == c
        mlo = m & (b_of == 0)
        mhi = m & (b_of == 1)
        seq_lo = np.zeros(NLOp * P, np.int16)
        seq_lo[sslot[mlo] * P + part[mlo]] = pos_lo[mlo].astype(np.int16)
        seq_hi = np.zeros(NHIp * P, np.int16)
        seq_hi[sslot[mhi] * P + part[mhi]] = pos_hi[mhi].astype(np.int16)
        seq = np.concatenate([seq_lo, seq_hi])
        # 16-partition wrap, replicated 8x: element i -> [i % 16, i // 16]
        idx16[c] = np.tile(seq.reshape(-1, 16).T, (8, 1))
        locv[c, part[m], wslot[m], 0] = (dd[m] - c * NS - w_of[m] * P).astype(
            np.float32
        )

    # per-core dinv layout [P, NW]: node c*NS + w*128 + p, padded with 1.0
    dinv_sb = np.ones((C, P, NW), np.float32)
    for c in range(C):
        dc = dinv[c * NS : (c + 1) * NS]
        dpad = np.concatenate([dc, np.ones(NW * P - NS, np.float32)])
        dinv_sb[c] = dpad.reshape(NW, P).T

    iota = np.zeros((P, 1, P), np.float32)
    iota[:, 0, :] = np.arange(P)[None, :]
    ident = np.eye(P, dtype=np.float32)

    # host-precomputed one-hot P (only for pres=True builds): pfull[c][p,
    # t*128+d] = (locv[c,p,t] == d)
    pfull = None
    if pres:
        ar = np.arange(P, dtype=np.float32)
        pfull = np.zeros((C, P, NTT * P), ml_dtypes.float8_e4m3)
        for c in range(C):
            oh = locv[c, :, :, 0][:, :, None] == ar[None, None, :]
            pfull[c] = oh.reshape(P, NTT * P).astype(ml_dtypes.float8_e4m3)

    shared = {
        "w1": W1e.astype(ml_dtypes.bfloat16),
        "w2": W2e.astype(ml_dtypes.bfloat16),
        "w3": W3e.astype(ml_dtypes.bfloat16),
        "b1r": np.tile(c1, (P, 1)).astype(np.float32),
        "b2r": np.tile(c2, (P, 1)).astype(np.float32),
        "b3r": np.tile(c3, (P, 1)).astype(np.float32),
        "iota": iota.astype(ml_dtypes.bfloat16),
        "ident": ident.astype(ml_dtypes.bfloat16),
    }
    xsc = (x * dinv[:, None]).astype(ml_dtypes.bfloat16)  # dinv pre-scale folded
    in_maps = []
    for c in range(C):
        m = dict(shared)
        # transposed [IN, NS] so stage A feeds matmul lhsT directly (no
        # on-device transpose)
        m["xst"] = np.ascontiguousarray(xsc[c * NS : (c + 1) * NS].T)
        m["idx16"] = np.ascontiguousarray(idx16[c])
        m["loc"] = np.ascontiguousarray(locv[c].astype(ml_dtypes.bfloat16))
        if pres:
            m["pfull"] = np.ascontiguousarray(pfull[c])
        m["dinv"] = np.ascontiguousarray(dinv_sb[c])
        in_maps.append(m)

    meta = dict(
        KA=KA.tolist(),
        KB=KB.tolist(),
        tile_base=tile_base.tolist(),
        lo_base=lo_base.tolist(),
        hi_base=hi_base.tolist(),
        NTT=NTT,
        NLOp=NLOp,
        NHIp=NHIp,
        chunk=chunk,
        perm=perm,
        noloop=noloop,
    )
    return in_maps, meta


# --------------------------------------------------------------------------
# device program
# --------------------------------------------------------------------------
def _build(meta, reps=1, stages="A1B2C3D", agg_mode="full", num_dev=C, ag_impl="cc",
           f8=True, scratch=None, pipe=2, pres=False, queues=4, spkt=True,
           gbufs=6, pabufs=2, barrier="", wpbufs=3, ppbufs=4, warm=False):
    KA, KB = meta["KA"], meta["KB"]
    tile_base, lo_base, hi_base = meta["tile_base"], meta["lo_base"], meta["hi_base"]
    NTT, NLOp, NHIp = meta["NTT"], meta["NLOp"], meta["NHIp"]
    noloop = meta.get("noloop", False)
    chunk = meta.get("chunk", CHUNK)
    NIDX = 8 * (NLOp + NHIp)
    HDT = F8 if f8 else BF  # H dtype for layers 1/2 (f8 gather row = 256B)

    nc = bacc.Bacc("TRN2", target_bir_lowering=False, debug=False, num_devices=num_dev,
                   dynamic_dma_scratch_size=scratch if scratch else 16384,
                   num_swdge_queues=queues)

    xs = nc.dram_tensor("xst", [IN, NS], BF, kind="ExternalInput")
    idx16 = nc.dram_tensor("idx16", [P, NIDX], I16, kind="ExternalInput")
    if pres:
        pfulld = nc.dram_tensor("pfull", [P, NTT * P], F8, kind="ExternalInput")
    else:
        locd = nc.dram_tensor("loc", [P, NTT, 1], BF, kind="ExternalInput")
    dinvd = nc.dram_tensor("dinv", [P, NW], F32, kind="ExternalInput")
    w1d = nc.dram_tensor("w1", [IN, HID], BF, kind="ExternalInput")
    w2d = nc.dram_tensor("w2", [HID, HID], BF, kind="ExternalInput")
    w3d = nc.dram_tensor("w3", [HID, OUTP], BF, kind="ExternalInput")
    b1d = nc.dram_tensor("b1r", [P, HID], F32, kind="ExternalInput")
    b2d = nc.dram_tensor("b2r", [P, HID], F32, kind="ExternalInput")
    b3d = nc.dram_tensor("b3r", [P, OUT], F32, kind="ExternalInput")
    iotad = nc.dram_tensor("iota", [P, 1, P], BF, kind="ExternalInput")
    identd = nc.dram_tensor("ident", [P, P], BF, kind="ExternalInput")
    outd = nc.dram_tensor("out", [NS, OUT], F32, kind="ExternalOutput")

    with tile.TileContext(nc) as tc:
        with (
            tc.tile_pool(name="const", bufs=1) as cp,
            tc.tile_pool(name="dram", bufs=1, space="DRAM") as dp,
            tc.tile_pool(name="work", bufs=wpbufs) as wp,
            tc.tile_pool(name="mlo", bufs=gbufs if chunk <= 8 else 2) as mplo,
            tc.tile_pool(name="mhi", bufs=gbufs if chunk <= 8 else 2) as mphi,
            tc.tile_pool(name="pwp", bufs=ppbufs) as pp,
            tc.tile_pool(name="hself", bufs=3) as hp,
            tc.tile_pool(name="accp", bufs=1) as acp,
            tc.tile_pool(name="smax", bufs=3) as sp,
            tc.tile_pool(name="ps_a", bufs=pabufs, space="PSUM") as ps_a,
            tc.tile_pool(name="ps_h", bufs=2, space="PSUM") as ps_h,
            tc.tile_pool(name="ps_t", bufs=2, space="PSUM") as ps_t,
        ):
            # ---- persistent constants in SBUF
            idx_sb = cp.tile([P, NIDX], I16, name="idx_sb", tag="idx_sb")
            nc.sync.dma_start(out=idx_sb[:], in_=idx16[:])
            if pres:
                pfull_sb = cp.tile([P, NTT * P], F8, name="pfull_sb", tag="pfull_sb")
                nc.sync.dma_start(out=pfull_sb[:], in_=pfulld[:])
            else:
                loc_sb = cp.tile([P, NTT, 1], BF, name="loc_sb", tag="loc_sb")
                nc.sync.dma_start(out=loc_sb[:], in_=locd[:])
                iota_sb = cp.tile([P, 1, P], BF, name="iota_sb", tag="iota_sb")
                nc.sync.dma_start(out=iota_sb[:], in_=iotad[:])
            dinv_sb = cp.tile([P, NW], F32, name="dinv_sb", tag="dinv_sb")
            nc.sync.dma_start(out=dinv_sb[:], in_=dinvd[:])
            ident_sb = cp.tile([P, P], BF, name="ident_sb", tag="ident_sb")
            nc.sync.dma_start(out=ident_sb[:], in_=identd[:])
            ident8_sb = None
            if noloop:
                ident8_sb = cp.tile([P, P], F8, name="ident8_sb", tag="ident8_sb")
                nc.scalar.copy(out=ident8_sb[:], in_=ident_sb[:])

            w_sb = {}
            for nm, dt_, dout in (("w1", w1d, HID), ("w2", w2d, HID), ("w3", w3d, OUTP)):
                t = cp.tile([P, 2 * dout], BF, name=f"{nm}_sb", tag=f"{nm}_sb")
                for kb in range(2):
                    nc.sync.dma_start(
                        out=t[:, kb * dout : (kb + 1) * dout],
                        in_=dt_[kb * P : (kb + 1) * P, :],
                    )
                w_sb[nm] = t
            b_sb = {}
            for nm, dt_, dout in (("b1", b1d, HID), ("b2", b2d, HID), ("b3", b3d, OUT)):
                t = cp.tile([P, dout], F32, name=f"{nm}_sb", tag=f"{nm}_sb")
                nc.sync.dma_start(out=t[:], in_=dt_[:])
                b_sb[nm] = t

            # ---- internal DRAM, split by row-block (lo = windows 0..23,
            # hi = 24..48) so each block AllGathers independently
            h_loc = {}
            h_full = {}
            for r in range(reps):
                for nm0, d, dt_ in (("h1", HID, HDT), ("h2", HID, HDT),
                                    ("h3", OUTP, BF)):
                    nm = f"{nm0}_{r}"
                    h_loc[nm] = (
                        dp.tile([GSPLIT, d], dt_, name=f"{nm}_llo", tag=f"{nm}_llo"),
                        dp.tile([NS - GSPLIT, d], dt_, name=f"{nm}_lhi",
                                tag=f"{nm}_lhi"),
                    )
                    h_full[nm] = (
                        dp.tile([BL0, d], dt_, name=f"{nm}_flo", tag=f"{nm}_flo",
                                addr_space="Shared" if ag_impl == "cc" else "Local"),
                        dp.tile([BL1, d], dt_, name=f"{nm}_fhi", tag=f"{nm}_fhi",
                                addr_space="Shared" if ag_impl == "cc" else "Local"),
                    )

            def h_loc_rows(h_loc_pair, w):
                """(tensor, row0) for window w's rows in the split h_loc."""
                if w < NWLO:
                    return h_loc_pair[0], w * P
                return h_loc_pair[1], (w - NWLO) * P

            def h_from_aT(m, rows, aTs, w_t, dout, h_loc_pair, hdt):
                ph = ps_h.tile([P, dout], F32, name="ph", tag="ph")
                for kb in range(2):
                    nc.tensor.matmul(
                        out=ph[:],
                        lhsT=aTs[kb][:],
                        rhs=w_t[:, kb * dout : (kb + 1) * dout],
                        start=(kb == 0),
                        stop=(kb == 1),
                    )
                h_t = wp.tile([P, dout], hdt, name="h_t", tag="h_t")
                nc.scalar.copy(out=h_t[:], in_=ph[:])
                ht_t, r0 = h_loc_rows(h_loc_pair, m)
                nc.sync.dma_start(out=ht_t[r0 : r0 + rows, :], in_=h_t[:rows, :])

            def h_stage(m, rows, act_ap, w_t, dout, h_loc_t, hdt):
                """act tile [P, 256] bf16 (node-major) -> H tile -> h_loc rows."""
                aTs = []
                for kb in range(2):
                    pt = ps_t.tile([P, P], BF, name=f"pt{kb}", tag=f"pt{kb}")
                    nc.tensor.transpose(
                        out=pt[:],
                        in_=act_ap[:, kb * P : (kb + 1) * P],
                        identity=ident_sb[:],
                    )
                    aT = wp.tile([P, P], BF, name=f"aT{kb}", tag=f"aT{kb}")
                    nc.scalar.copy(out=aT[:], in_=pt[:])
                    aTs.append(aT)
                h_from_aT(m, rows, aTs, w_t, dout, h_loc_t, hdt)

            def allgather(nm, part):
                nc.gpsimd.collective_compute(
                    "AllGather",
                    AX.bypass,
                    replica_groups=[list(range(C))],
                    ins=[h_loc[nm][part][:].opt()],
                    outs=[h_full[nm][part][:].opt()],
                )

            def agg_stage(h_full_pair, elem, dagg, bias_t, w_next, dnext,
                          h_next_pair, last, mdt, hdt_next=BF, h_self_pair=None,
                          nm_next=None):
                """Two-pass aggregation: pass 0 consumes the lo-block gather
                stream (+ self rows) into SBUF accumulators — it only needs
                AG-lo, so it runs while AG-hi is still on the wire; pass 1
                consumes the hi stream, combines, and postprocesses. The next
                layer's AG-lo fires mid-pass-1 (after window 23's conv)."""
                mode = agg_mode
                selfadd = noloop and mode in ("full", "nopost")
                lo_ch = {}
                hi_ch = {}
                issued = {"lo": 0, "hi": 0}

                def issue(stream, cid):
                    pool = mplo if stream == "lo" else mphi
                    base_col = 0 if stream == "lo" else 8 * NLOp
                    t = pool.tile(
                        [P, chunk, elem], mdt, name=f"m{stream}", tag=f"m{stream}"
                    )
                    # each pass drives a single stream, so spread that stream
                    # over ALL queues (odd/even split would idle half of them)
                    qn = cid % queues
                    nc.gpsimd.dma_gather(
                        out_ap=t[:, :, :],
                        in_ap=h_full_pair[0][:, :] if stream == "lo"
                        else h_full_pair[1][:, :],
                        idxs_ap=idx_sb[
                            :, base_col + 8 * chunk * cid : base_col + 8 * chunk * (cid + 1)
                        ],
                        num_idxs=chunk * P,
                        num_idxs_reg=chunk * P,
                        elem_size=elem,
                        queue_num=qn,
                        single_packet=spkt,
                    )
                    (lo_ch if stream == "lo" else hi_ch)[cid] = t

                # ---- pass 0: lo-block tiles + self rows -> SBUF accumulators
                accs = {}
                for w in range(NW):
                    rows = min(P, NS - w * P)
                    tb = int(tile_base[w])
                    ka = int(KA[w])
                    if ka:
                        need = (int(lo_base[w]) + ka - 1) // chunk
                        while issued["lo"] <= need:
                            issue("lo", issued["lo"])
                            issued["lo"] += 1
                    if mode == "gather":
                        continue
                    if ka:
                        Pw = pp.tile([P, ka, P], mdt, name="Pwl", tag="Pwl")
                        nc.vector.tensor_tensor(
                            out=Pw[:],
                            in0=loc_sb[:, tb : tb + ka, :1].to_broadcast([P, ka, P]),
                            in1=iota_sb[:].to_broadcast([P, ka, P]),
                            op=AX.is_equal,
                        )
                    if mode == "ponly":
                        continue
                    nmm = ka + (1 if selfadd else 0)
                    acc = acp.tile([P, dagg], F32, name=f"acc{w}",
                                   tag=f"acc{w}_{dagg}")
                    if nmm == 0:
                        nc.vector.memset(acc[:], 0.0)
                        accs[w] = acc
                        continue
                    pa = ps_a.tile([P, dagg], F32, name="pa", tag="pa")
                    if selfadd:
                        # self-loop contribution: contiguous local rows, no
                        # gather descriptors — identity matmul from h_loc
                        hs = hp.tile([P, dagg], mdt, name="hs", tag="hs")
                        if rows < P:
                            nc.vector.memset(hs[:], 0.0)
                        st_t, r0 = h_loc_rows(h_self_pair, w)
                        nc.sync.dma_start(
                            out=hs[:rows, :],
                            in_=st_t[r0 : r0 + rows, :dagg],
                        )
                        nc.tensor.matmul(
                            out=pa[:],
                            lhsT=ident8_sb[:] if mdt == F8 else ident_sb[:],
                            rhs=hs[:],
                            start=True,
                            stop=(ka == 0),
                        )
                    for kk in range(ka):
                        sid = int(lo_base[w]) + kk
                        t = lo_ch[sid // chunk]
                        nc.tensor.matmul(
                            out=pa[:],
                            lhsT=Pw[:, kk, :],
                            rhs=t[:, sid % chunk, :dagg],
                            start=(kk == 0 and not selfadd),
                            stop=(kk == ka - 1),
                        )
                    nc.scalar.copy(out=acc[:], in_=pa[:])
                    accs[w] = acc

                # ---- pass 1: hi-block tiles + acc -> post (+ next-layer AGs)
                for w in range(NW):
                    rows = min(P, NS - w * P)
                    tb = int(tile_base[w])
                    ka = int(KA[w])
                    kb_ = int(KB[w])
                    if kb_:
                        need = (int(hi_base[w]) + kb_ - 1) // chunk
                        while issued["hi"] <= need:
                            issue("hi", issued["hi"])
                            issued["hi"] += 1
                    if mode == "gather":
                        continue
                    if kb_:
                        Pw = pp.tile([P, kb_, P], mdt, name="Pwh", tag="Pwh")
                        nc.vector.tensor_tensor(
                            out=Pw[:],
                            in0=loc_sb[:, tb + ka : tb + ka + kb_, :1].to_broadcast(
                                [P, kb_, P]
                            ),
                            in1=iota_sb[:].to_broadcast([P, kb_, P]),
                            op=AX.is_equal,
                        )
                    if mode == "ponly":
                        continue
                    if kb_:
                        pa = ps_a.tile([P, dagg], F32, name="pa", tag="pa")
                        for kk in range(kb_):
                            sid = int(hi_base[w]) + kk
                            t = hi_ch[sid // chunk]
                            nc.tensor.matmul(
                                out=pa[:],
                                lhsT=Pw[:, kk, :],
                                rhs=t[:, sid % chunk, :dagg],
                                start=(kk == 0),
                                stop=(kk == kb_ - 1),
                            )
                    if mode == "nopost":
                        continue
                    if kb_:
                        comb = wp.tile([P, dagg], F32, name="comb", tag="comb")
                        nc.vector.tensor_tensor(
                            out=comb[:], in0=pa[:], in1=accs[w][:], op=AX.add
                        )
                        src_ap = comb[:]
                    else:
                        src_ap = accs[w][:]
                    if not last:
                        t1 = wp.tile([P, dagg], F32, name="t1", tag="t1")
                        nc.vector.scalar_tensor_tensor(
                            out=t1[:],
                            in0=src_ap,
                            scalar=dinv_sb[:, w : w + 1],
                            in1=bias_t[:],
                            op0=AX.mult,
                            op1=AX.add,
                        )
                        act_t = wp.tile([P, dagg], BF, name="act_t", tag="act_t")
                        nc.scalar.activation(
                            out=act_t[:],
                            in_=t1[:],
                            func=AF.Relu,
                            scale=dinv_sb[:, w : w + 1],
                        )
                        h_stage(w, rows, act_t[:], w_next, dnext, h_next_pair,
                                hdt_next)
                        if nm_next is not None:
                            if w == NWLO - 1:
                                allgather(nm_next, 0)
                            elif w == NW - 1:
                                allgather(nm_next, 1)
                        continue
                    t1 = sp.tile([P, OUT], F32, name="t1s", tag="t1s")
                    nc.vector.scalar_tensor_tensor(
                        out=t1[:],
                        in0=src_ap,
                        scalar=dinv_sb[:, w : w + 1],
                        in1=bias_t[:],
                        op0=AX.mult,
                        op1=AX.add,
                    )
                    if True:
                        negm = sp.tile([P, 1], F32, name="negm", tag="negm")
                        nc.vector.tensor_reduce(
                            out=negm[:],
                            in_=t1[:],
                            axis=mybir.AxisListType.X,
                            op=AX.max,
                            negate=True,
                        )
                        ex = sp.tile([P, OUT], F32, name="ex", tag="ex")
                        ssum = sp.tile([P, 1], F32, name="ssum", tag="ssum")
                        nc.scalar.activation(
                            out=ex[:],
                            in_=t1[:],
                            func=AF.Exp,
                            bias=negm[:],
                            accum_out=ssum[:],
                        )
                        rinv = sp.tile([P, 1], F32, name="rinv", tag="rinv")
                        nc.vector.reciprocal(out=rinv[:], in_=ssum[:])
                        o = sp.tile([P, OUT], F32, name="o", tag="o")
                        nc.vector.tensor_scalar_mul(out=o[:], in0=ex[:], scalar1=rinv[:])
                        nc.sync.dma_start(
                            out=outd[w * P : w * P + rows, :], in_=o[:rows, :]
                        )

            def stage_A(r):
                h1 = f"h1_{r}"
                for m in range(NW):
                    rows = min(P, NS - m * P)
                    aTs = []
                    for kb in range(2):
                        aT = wp.tile([P, P], BF, name=f"aT{kb}", tag=f"aT{kb}")
                        if rows < P:
                            nc.vector.memset(aT[:], 0.0)
                        nc.sync.dma_start(
                            out=aT[:, :rows],
                            in_=xs[kb * P : (kb + 1) * P, m * P : m * P + rows],
                        )
                        aTs.append(aT)
                    h_from_aT(m, rows, aTs, w_sb["w1"], HID, h_loc[h1], HDT)
                    if "1" in stages:
                        if m == NWLO - 1:
                            allgather(h1, 0)
                        elif m == NW - 1:
                            allgather(h1, 1)

            def stage_B(r):
                agg_stage(
                    h_full[f"h1_{r}"], HID, HID, b_sb["b1"], w_sb["w2"], HID,
                    h_loc[f"h2_{r}"], last=False, mdt=HDT, hdt_next=HDT,
                    h_self_pair=h_loc[f"h1_{r}"],
                    nm_next=f"h2_{r}" if "2" in stages else None,
                )

            def stage_C(r):
                agg_stage(
                    h_full[f"h2_{r}"], HID, HID, b_sb["b2"], w_sb["w3"], OUTP,
                    h_loc[f"h3_{r}"], last=False, mdt=HDT, hdt_next=BF,
                    h_self_pair=h_loc[f"h2_{r}"],
                    nm_next=f"h3_{r}" if "3" in stages else None,
                )

            def stage_D(r):
                agg_stage(
                    h_full[f"h3_{r}"], OUTP, OUT, b_sb["b3"], None, 0, None,
                    last=True, mdt=BF, h_self_pair=h_loc[f"h3_{r}"],
                )

            def swdge_warm():
                # one tiny gather per queue: absorbs first-use SWDGE ring
                # latency under stage A instead of stage B's stream head
                wrmi = cp.tile([P, 8], I16, name="wrmi", tag="wrmi")
                nc.vector.memset(wrmi[:], 0)
                for q in range(queues):
                    wt = wp.tile([P, 1, P], I16, name="wrm_t", tag="wrm_t")
                    nc.gpsimd.dma_gather(
                        out_ap=wt[:, :, :],
                        in_ap=idx16[:, 0:P],
                        idxs_ap=wrmi[:, 0:8],
                        num_idxs=P,
                        num_idxs_reg=P,
                        elem_size=P,
                        elem_step=NIDX,
                        queue_num=q,
                        single_packet=spkt,
                    )

            # AllGathers are embedded in the producing stage (after windows 23
            # and 48); the "1"/"2"/"3" stage letters only gate whether they
            # are emitted.
            emit = {
                "A": stage_A,
                "1": lambda r: None,
                "B": stage_B,
                "2": lambda r: None,
                "C": stage_C,
                "3": lambda r: None,
                "D": stage_D,
            }
            # pair-interleaved emission: within a pair of reps, emit each
            # stage for both reps before moving on, so rep r's collective
            # overlaps rep r^1's compute in every engine's program order.
            for base in range(0, reps, pipe):
                pair = [base + j for j in range(pipe) if base + j < reps]
                if warm:
                    swdge_warm()
                for st in stages:
                    for r in pair:
                        emit[st](r)
                    if "s" in barrier:
                        tc.strict_bb_all_engine_barrier()
                if "r" in barrier:
                    tc.strict_bb_all_engine_barrier()

    nc.compile()
    return nc


# --------------------------------------------------------------------------
# persistent-staging runner (inputs stay device-resident between calls)
# --------------------------------------------------------------------------
def _make_runner(nc, in_maps):
    import jax
    from jax.experimental.shard_map import shard_map
    from jax.sharding import Mesh, NamedSharding, PartitionSpec

    from concourse import bass2jax, mybir as mb

    bass2jax.install_neuronx_cc_hook()

    in_names, out_names, out_avals, zero_shapes = [], [], [], []
    for alloc in nc.m.functions[0].allocations:
        if not isinstance(alloc, mb.MemoryLocationSet):
            continue
        name = alloc.memorylocations[0].name
        if alloc.kind == "ExternalInput":
            in_names.append(name)
        elif alloc.kind == "ExternalOutput":
            out_names.append(name)
            shape = tuple(alloc.tensor_shape)
            dtype = mb.dt.np(alloc.dtype)
            out_avals.append(jax.core.ShapedArray(shape, dtype))
            zero_shapes.append((shape, dtype))
    part_name = nc.partition_id_tensor.name if nc.partition_id_tensor else None
    if part_name is not None and part_name in in_names:
        in_names.remove(part_name)
    n_params = len(in_names)
    n_outs = len(out_names)
    all_names = in_names + out_names + ([part_name] if part_name else [])

    def _body(*args):
        operands = list(args)
        if part_name is not None:
            operands.append(bass2jax.partition_id_tensor())
        outs = bass2jax._bass_exec_p.bind(
            *operands,
            out_avals=tuple(out_avals),
            in_names=tuple(all_names),
            out_names=tuple(out_names),
            lowering_input_output_aliases=(),
            sim_require_finite=True,
            sim_require_nnan=True,
            nc=nc,
        )
        return tuple(outs)

    devices = jax.devices()[:C]
    mesh = Mesh(np.asarray(devices), ("core",))
    in_specs = (PartitionSpec("core"),) * (n_params + n_outs)
    out_specs = (PartitionSpec("core"),) * n_outs
    donate = tuple(range(n_params, n_params + n_outs))
    sharded = jax.jit(
        shard_map(_body, mesh=mesh, in_specs=in_specs, out_specs=out_specs,
                  check_rep=False),
        donate_argnums=donate,
        keep_unused=True,
    )
    sh = NamedSharding(mesh, PartitionSpec("core"))
    in_dev = [
        jax.device_put(
            np.concatenate([np.asarray(in_maps[c][n]) for c in range(C)], axis=0), sh
        )
        for n in in_names
    ]
    import jax.numpy as jnp

    zeros_fn = jax.jit(
        lambda: tuple(
            jnp.zeros((C * s[0], *s[1:]), d) for s, d in zero_shapes
        ),
        out_shardings=tuple(sh for _ in zero_shapes),
    )

    def run(fetch=True):
        zeros = zeros_fn()
        outs = sharded(*in_dev, *zeros)
        jax.block_until_ready(outs)
        if not fetch:
            return None
        return [
            {
                n: np.asarray(outs[i]).reshape(C, *out_avals[i].shape)[c]
                for i, n in enumerate(out_names)
            }
            for c in range(C)
        ]

    return run


# --------------------------------------------------------------------------
# entry points
# --------------------------------------------------------------------------
def _execute(inputs, reps=1, runs=1):
    in_maps, meta = _host_prep(inputs)
    t0 = time.time()
    nc = _build(meta, reps=reps, scratch=65536, pipe=2, queues=4, gbufs=12)
    t1 = time.time()
    walls = []
    res = None
    for _ in range(runs):
        ts = time.time()
        res = bass_utils.run_bass_kernel_spmd(nc, in_maps, list(range(C)))
        walls.append(time.time() - ts)
    out_new = np.concatenate([res.results[c]["out"] for c in range(C)], axis=0)
    out = np.empty_like(out_new)
    out[meta["perm"]] = out_new
    return out, dict(build_s=t1 - t0, walls=walls)


def kernel(**inputs) -> np.ndarray:
    out, _ = _execute(inputs, reps=1, runs=1)
    return out.astype(np.float32)


if __name__ == "__main__":
    rng = np.random.default_rng(0)
    d = {
        "x": rng.standard_normal((N, IN)).astype(np.float32),
        "edge_index": rng.integers(0, N, size=(2, E)).astype(np.int32),
    }
    for i, (di, do) in enumerate(((IN, HID), (HID, HID), (HID, OUT)), 1):
        d[f"W{i}"] = (rng.standard_normal((di, do)) * 0.05).astype(np.float32)
        d[f"b{i}"] = np.zeros(do, np.float32)
        if i < 3:
            d[f"g{i}"] = np.ones(do, np.float32)
            d[f"beta{i}"] = np.zeros(do, np.float32)
            d[f"m{i}"] = (rng.standard_normal(do) * 0.1).astype(np.float32)
            d[f"v{i}"] = rng.uniform(0.5, 1.5, do).astype(np.float32)
    out, info = _execute(d)
    print("out shape:", out.shape, "info:", info)



# revision 32
# speedup vs baseline: 1.0559x; 1.0559x over previous
"""3-layer GCN (GCNConv+BN+ReLU x2, GCNConv, softmax) on 8 Trainium2 NeuronCores.

Strategy (graph/data parallel, per sharding hint):
  - Nodes sharded 6250/core. Edges partitioned by destination core, sorted by
    dst, grouped into 128-dst windows, padded to 128-edge tiles.
  - Symmetric normalization coef = dinv[src]*dinv[dst] is factored into
    per-node pre/post scaling (dinv; the input-side factor is folded into x
    on the host), so per-edge work is a pure row gather + one-hot matmul.
  - BatchNorm (eval) folded into the conv weights/bias on the host. x is
    shipped pre-transposed [IN, NS] so stage A feeds matmul lhsT directly.
  - H1/H2 stored fp8e4 (256B rows = min gather elem; halves AG wire + gather
    bytes; rel err stays ~2.6e-4 vs the 2e-2 gate). H3 stays bf16x128.
  - Per layer: H_local = act_local @ W (TensorE), AllGather H (on one chip
    the collective is nearly free), then per dst-window: dma_gather rows of
    H, build one-hot P via iota is_equal (VectorE), accumulate P.T @ msg
    into PSUM (TensorE), postprocess (bias/relu/dinv scaling).
  - SELF-LOOPS bypass the gather stream entirely (noloop=True): each
    window's self contribution is its own contiguous h_loc rows, added via
    one identity matmul from an HWDGE dma_start — zero SWDGE descriptors.
  - Node permutation (balance=True): nodes are pair-dealt (pairs matched by
    opposite lo/hi indegree skew so every pair is bucket-neutral, snaked by
    total weight) + swap-repaired so per-(window,bucket) indegree maxima
    over cores sit under the 128-row tile boundary. With self-loops removed
    this cuts the padded tile count NTT ~967 -> ~797 (-18% descriptors).
  - BLOCK-SPLIT HALO EXCHANGE: each layer's H is stored as two row-blocks
    (windows 0..23 of every core = block-lo [8x3072], windows 24..48 =
    block-hi [8x3178], both int16-addressable) that AllGather
    independently. AG-lo fires right after window 23's conv — mid-stage —
    and AG-hi at stage end, so AG wire hides under gather streams.
    Aggregation is two-pass: pass 0 consumes the lo-block gather stream (+
    self rows) into per-window SBUF f32 accumulators while AG-hi is still
    on the wire; pass 1 consumes the hi stream, combines, postprocesses,
    and feeds the next conv. Each pass spreads its single stream across
    ALL 4 SWDGE queues.
  - Gathers use int16 indices into the block tables, fetched in 8-tile
    chunks (1024 indices/call — the HW limit for one SWDGE gather; 2048
    wedges the device).
  - PERF-CRITICAL: gathers are SWDGE-drain bound (~2.9us/1024x256B call at 4
    queues = per-queue DMA engine bandwidth); num_swdge_queues=4 is the
    ucode max; dynamic_dma_scratch_size=65536 keeps the per-queue descriptor
    rings from serializing desc-gen against drain. The gather streams are
    ~85% of the serialized per-rep makespan.
  - Final layer: width-64 aggregation + softmax, output f32 shards.
Measured (serialized rep slope, inter-rep barrier): baseline 1.67 ->
1.20 (noloop+balance+xsT, single-pass) -> ~0.95-1.02ms/rep (block-split,
gbufs=12).
"""
import sys
import time

sys.path.insert(0, "/opt/trn_rl_repo")

import numpy as np
import ml_dtypes

from concourse import bass, mybir, bacc, tile
from concourse import bass_utils

# problem constants (hardcoded per contract)
N, E = 50000, 800000
IN = 256
HID = 256
OUT = 64
OUTP = 128  # padded width for layer-3 H (dma_gather needs >=256B rows)
C = 8
NS = N // C  # 6250 nodes per core
P = 128
NW = (NS + P - 1) // P  # 49 windows per core
SPLIT = 25000  # src-index split so indices fit int16
EPS = 1e-5
CHUNK = 8  # tiles per dma_gather call (8*128 = 1024 idx = HW limit)
# block split: local rows [0, GSPLIT) = windows 0..23 form block-lo across all
# cores (8*3072 = 24576 rows, int16-addressable); rows [GSPLIT, NS) = windows
# 24..48 form block-hi (8*3178 = 25424 rows). Each block AllGathers
# independently, so AG-lo fires after window 23 and AG wire hides under the
# gather streams.
GSPLIT = 24 * P  # 3072
NWLO = 24
BL0 = C * GSPLIT  # 24576
BL1 = C * (NS - GSPLIT)  # 25424

BF = mybir.dt.bfloat16
F8 = mybir.dt.float8e4
F32 = mybir.dt.float32
I16 = mybir.dt.int16

AX = mybir.AluOpType
AF = mybir.ActivationFunctionType


# --------------------------------------------------------------------------
# host-side preprocessing
# --------------------------------------------------------------------------
def _balance_perm(a_in, b_in):
    """Permute nodes (within src row-blocks) so per-(window,bucket) in-degree
    sums are balanced across cores: pair nodes with opposite (a-b) skew so
    each pair is bucket-neutral, snake-deal pairs by total weight over the
    block's windows, then swap-repair windows whose max-over-core sum sits
    just above a 128-row tile boundary. perm[new_id] = old_id."""
    perm = np.empty(N, np.int64)
    allpos = np.arange(N)
    groups = [
        (allpos[(allpos % NS) < GSPLIT], range(0, NWLO)),
        (allpos[(allpos % NS) >= GSPLIT], range(NWLO, NW)),
    ]
    for nodes, wrange in groups:
        o = nodes[np.argsort(a_in[nodes] - b_in[nodes], kind="stable")]
        half = len(o) // 2
        first, second = o[:half], o[half:][::-1]
        pw = a_in[first] + b_in[first] + a_in[second] + b_in[second]
        po = np.argsort(-pw, kind="stable")
        first, second = first[po], second[po]
        wins = []
        for c in range(C):
            for wl_ in wrange:
                base = c * NS + wl_ * P
                wins.append((base, min(P, NS - wl_ * P)))
        bases = np.array([b for b, _ in wins])
        caps = np.array([cp for _, cp in wins])
        fill = np.zeros(len(wins), np.int64)
        pos, r = 0, 0
        while pos < half:
            active = np.where(fill + 2 <= caps)[0]
            if len(active) == 0:
                break
            if r % 2 == 1:
                active = active[::-1]
            take = min(len(active), half - pos)
            sel = active[:take]
            perm[bases[sel] + fill[sel]] = first[pos : pos + take]
            perm[bases[sel] + fill[sel] + 1] = second[pos : pos + take]
            fill[sel] += 2
            pos += take
            r += 1
        rem = list(first[pos:]) + list(second[pos:])
        ri = 0
        for j in np.where(fill < caps)[0]:
            while fill[j] < caps[j]:
                perm[bases[j] + fill[j]] = rem[ri]
                ri += 1
                fill[j] += 1
        assert ri == len(rem)
    return perm


def _repair_perm(perm, a_in, b_in, iters=24):
    """Greedy swap-repair on the permuted layout: for (window,bucket) cells
    whose max-over-core sum is just over a tile boundary, swap a heavy node
    out (within the same core) for a light one from a window with headroom."""
    aw = np.stack([a_in, b_in], 1)  # [N,2]

    def cells(perm):
        # cnt[c,w,b] = sum of bucket-b indegree of nodes in (c,w)
        cnt = np.zeros((C, NW, 2), np.int64)
        for c in range(C):
            for w in range(NW):
                sl = perm[c * NS + w * P : c * NS + w * P + min(P, NS - w * P)]
                cnt[c, w, 0] = a_in[sl].sum()
                cnt[c, w, 1] = b_in[sl].sum()
        return cnt

    cnt = cells(perm)
    for _ in range(iters):
        K = np.ceil(cnt.max(axis=0) / P).astype(np.int64)  # [NW,2]
        ntt0 = int(K.sum())
        # candidate cells: overflow above (K-1)*P boundary, smallest first
        cand = []
        for w in range(NW):
            for b in range(2):
                lim = (K[w, b] - 1) * P
                gap = int(cnt[:, w, b].max() - lim)
                if 0 < gap <= 64:
                    cand.append((gap, w, b))
        if not cand:
            break
        cand.sort()
        moved = 0
        for gap, w, b in cand[:40]:
            lim = (K[w, b] - 1) * P
            for c in np.where(cnt[:, w, b] > lim)[0]:
                sl_lo = c * NS + w * P
                nrow = min(P, NS - w * P)
                sl = perm[sl_lo : sl_lo + nrow]
                # try donor windows on same core AND same row-block (swaps
                # must not move nodes across the block boundary) with
                # headroom in BOTH buckets
                for w2 in np.argsort(cnt[c, :, b]):
                    if w2 == w or (w2 < NWLO) != (w < NWLO):
                        continue
                    need = int(cnt[c, w, b] - lim)
                    if need <= 0:
                        break
                    s2_lo = c * NS + w2 * P
                    nrow2 = min(P, NS - w2 * P)
                    sl2 = perm[s2_lo : s2_lo + nrow2]
                    d1 = aw[sl, b]
                    d2 = aw[sl2, b]
                    u_i = int(np.argmax(d1))
                    v_i = int(np.argmin(d2))
                    delta = int(d1[u_i] - d2[v_i])
                    if delta <= 0:
                        continue
                    # other-bucket delta
                    ob = 1 - b
                    delta_o = int(aw[sl[u_i], ob] - aw[sl2[v_i], ob])
                    # check donor window stays under ITS boundaries
                    if cnt[c, w2, b] + delta > K[w2, b] * P:
                        continue
                    if cnt[c, w2, ob] + (-delta_o) > K[w2, ob] * P:
                        continue
                    if cnt[c, w, ob] + delta_o > K[w, ob] * P:
                        continue
                    u, v = sl[u_i], sl2[v_i]
                    perm[sl_lo + u_i], perm[s2_lo + v_i] = v, u
                    cnt[c, w, b] -= delta
                    cnt[c, w2, b] += delta
                    cnt[c, w, ob] -= delta_o
                    cnt[c, w2, ob] += delta_o
                    sl = perm[sl_lo : sl_lo + nrow]
                    moved += 1
        Knew = np.ceil(cnt.max(axis=0) / P).astype(np.int64)
        if int(Knew.sum()) >= ntt0 and moved == 0:
            break
    return perm


def _host_prep(inputs, chunk=CHUNK, balance=True, noloop=True, srcsort=True,
               pres=False):
    x = np.asarray(inputs["x"], np.float32)
    ei = np.asarray(inputs["edge_index"]).astype(np.int64)
    W1 = np.asarray(inputs["W1"], np.float32)
    b1 = np.asarray(inputs["b1"], np.float32)
    g1 = np.asarray(inputs["g1"], np.float32)
    beta1 = np.asarray(inputs["beta1"], np.float32)
    m1 = np.asarray(inputs["m1"], np.float32)
    v1 = np.asarray(inputs["v1"], np.float32)
    W2 = np.asarray(inputs["W2"], np.float32)
    b2 = np.asarray(inputs["b2"], np.float32)
    g2 = np.asarray(inputs["g2"], np.float32)
    beta2 = np.asarray(inputs["beta2"], np.float32)
    m2 = np.asarray(inputs["m2"], np.float32)
    v2 = np.asarray(inputs["v2"], np.float32)
    W3 = np.asarray(inputs["W3"], np.float32)
    b3 = np.asarray(inputs["b3"], np.float32)

    # gather-stream edge set: self-loops handled separately (noloop) via a
    # contiguous per-window identity add from h_loc — no gather descriptors.
    loops = np.arange(N, dtype=np.int64)
    if noloop:
        src = ei[0].copy()
        dst = ei[1].copy()
    else:
        src = np.concatenate([ei[0], loops])
        dst = np.concatenate([ei[1], loops])

    # Node permutation, constrained to permute within each src row-block (so
    # each node's lo/hi source-bucket is preserved): balances each window's
    # (lo-indegree, hi-indegree) jointly, shrinking the max-over-cores
    # ceil(cnt/128) tile padding. perm[new_id] = old_id.
    a_in = np.bincount(dst[(src % NS) < GSPLIT], minlength=N).astype(np.int64)
    b_in = np.bincount(dst[(src % NS) >= GSPLIT], minlength=N).astype(np.int64)

    if balance:
        perm = _balance_perm(a_in, b_in)
        perm = _repair_perm(perm, a_in, b_in)
    else:
        perm = np.arange(N, dtype=np.int64)
    inv = np.empty(N, np.int64)
    inv[perm] = np.arange(N)

    x = x[perm]
    src = inv[src]
    dst = inv[dst]

    # degree INCLUDES the self-loop (reference semantics) regardless of noloop
    deg = np.bincount(dst, minlength=N).astype(np.float32)
    if noloop:
        deg = deg + 1.0
    dinv = (1.0 / np.sqrt(np.maximum(deg, 1.0))).astype(np.float32)

    # fold BN into conv weights/bias
    s1 = g1 / np.sqrt(v1 + EPS)
    W1e = W1 * s1[None, :]
    c1 = (b1 - m1) * s1 + beta1
    s2 = g2 / np.sqrt(v2 + EPS)
    W2e = W2 * s2[None, :]
    c2 = (b2 - m2) * s2 + beta2
    W3e = np.concatenate([W3, np.zeros((HID, OUTP - OUT), np.float32)], axis=1)
    c3 = b3

    # edge grouping: (owner core, window, lo/hi src row-block)
    owner = dst // NS
    wl = (dst % NS) // P
    bkt = ((src % NS) >= GSPLIT).astype(np.int64)
    grp = (owner * NW + wl) * 2 + bkt
    # NOTE: src-sorting within groups was slower on the old monolithic h_full
    # layout (correlated ascending streams from 8 cores contending on HBM
    # banks) but measured ~4-8% FASTER on the per-block tables.
    order = np.lexsort((src, grp)) if srcsort else np.argsort(grp, kind="stable")
    g = grp[order]
    ss = src[order]
    dd = dst[order]

    cnt = np.bincount(g, minlength=C * NW * 2)
    cnt3 = cnt.reshape(C, NW, 2)
    KA = np.ceil(cnt3[:, :, 0].max(axis=0) / P).astype(np.int64)  # [NW]
    KB = np.ceil(cnt3[:, :, 1].max(axis=0) / P).astype(np.int64)
    K = KA + KB
    # window-contiguous tile order (for loc / P / matmuls)
    tile_base = np.concatenate([[0], np.cumsum(K)]).astype(np.int64)
    NTT = int(K.sum())
    # bucket stream order (for gather chunks)
    lo_base = np.concatenate([[0], np.cumsum(KA)]).astype(np.int64)
    hi_base = np.concatenate([[0], np.cumsum(KB)]).astype(np.int64)
    NLO = int(KA.sum())
    NHI = int(KB.sum())
    NLOp = (NLO + chunk - 1) // chunk * chunk
    NHIp = (NHI + chunk - 1) // chunk * chunk

    starts = np.concatenate([[0], np.cumsum(cnt)]).astype(np.int64)
    rank = np.arange(g.size, dtype=np.int64) - starts[g]
    ow = g >> 1
    c_of = ow // NW
    w_of = ow % NW
    b_of = g & 1
    # stream slot (gather order): position within lo/hi stream
    sslot = np.where(
        b_of == 0, lo_base[w_of] + rank // P, hi_base[w_of] + rank // P
    )
    # window slot (matmul/P order)
    wslot = tile_base[w_of] + np.where(b_of == 0, 0, KA[w_of]) + rank // P
    part = rank % P

    # source position within its block table: block-lo rows are laid out
    # core-major [core*GSPLIT + r], block-hi [core*(NS-GSPLIT) + (r-GSPLIT)]
    s_core = ss // NS
    s_r = ss % NS
    pos_lo = s_core * GSPLIT + s_r
    pos_hi = s_core * (NS - GSPLIT) + (s_r - GSPLIT)

    idx16 = np.zeros((C, P, 8 * (NLOp + NHIp)), np.int16)
    locv = np.full((C, P, NTT, 1), -1.0, np.float32)
    for c in range(C):
        m = c_of == c
        mlo = m & (b_of == 0)
        mhi = m & (b_of == 1)
        seq_lo = np.zeros(NLOp * P, np.int16)
        seq_lo[sslot[mlo] * P + part[mlo]] = pos_lo[mlo].astype(np.int16)
        seq_hi = np.zeros(NHIp * P, np.int16)
        seq_hi[sslot[mhi] * P + part[mhi]] = pos_hi[mhi].astype(np.int16)
        seq = np.concatenate([seq_lo, seq_hi])
        # 16-partition wrap, replicated 8x: element i -> [i % 16, i // 16]
        idx16[c] = np.tile(seq.reshape(-1, 16).T, (8, 1))
        locv[c, part[m], wslot[m], 0] = (dd[m] - c * NS - w_of[m] * P).astype(
            np.float32
        )

    # per-core dinv layout [P, NW]: node c*NS + w*128 + p, padded with 1.0
    dinv_sb = np.ones((C, P, NW), np.float32)
    for c in range(C):
        dc = dinv[c * NS : (c + 1) * NS]
        dpad = np.concatenate([dc, np.ones(NW * P - NS, np.float32)])
        dinv_sb[c] = dpad.reshape(NW, P).T

    iota = np.zeros((P, 1, P), np.float32)
    iota[:, 0, :] = np.arange(P)[None, :]
    ident = np.eye(P, dtype=np.float32)

    # host-precomputed one-hot P (only for pres=True builds): pfull[c][p,
    # t*128+d] = (locv[c,p,t] == d)
    pfull = None
    if pres:
        ar = np.arange(P, dtype=np.float32)
        pfull = np.zeros((C, P, NTT * P), ml_dtypes.float8_e4m3)
        for c in range(C):
            oh = locv[c, :, :, 0][:, :, None] == ar[None, None, :]
            pfull[c] = oh.reshape(P, NTT * P).astype(ml_dtypes.float8_e4m3)

    shared = {
        "w1": W1e.astype(ml_dtypes.bfloat16),
        "w2": W2e.astype(ml_dtypes.bfloat16),
        "w3": W3e.astype(ml_dtypes.bfloat16),
        "b1r": np.tile(c1, (P, 1)).astype(np.float32),
        "b2r": np.tile(c2, (P, 1)).astype(np.float32),
        "b3r": np.tile(c3, (P, 1)).astype(np.float32),
        "iota": iota.astype(ml_dtypes.bfloat16),
        "ident": ident.astype(ml_dtypes.bfloat16),
    }
    xsc = (x * dinv[:, None]).astype(ml_dtypes.bfloat16)  # dinv pre-scale folded
    in_maps = []
    for c in range(C):
        m = dict(shared)
        # transposed [IN, NS] so stage A feeds matmul lhsT directly (no
        # on-device transpose)
        m["xst"] = np.ascontiguousarray(xsc[c * NS : (c + 1) * NS].T)
        m["idx16"] = np.ascontiguousarray(idx16[c])
        m["loc"] = np.ascontiguousarray(locv[c].astype(ml_dtypes.bfloat16))
        if pres:
            m["pfull"] = np.ascontiguousarray(pfull[c])
        m["dinv"] = np.ascontiguousarray(dinv_sb[c])
        in_maps.append(m)

    meta = dict(
        KA=KA.tolist(),
        KB=KB.tolist(),
        tile_base=tile_base.tolist(),
        lo_base=lo_base.tolist(),
        hi_base=hi_base.tolist(),
        NTT=NTT,
        NLOp=NLOp,
        NHIp=NHIp,
        chunk=chunk,
        perm=perm,
        noloop=noloop,
    )
    return in_maps, meta


# --------------------------------------------------------------------------
# device program
# --------------------------------------------------------------------------
def _build(meta, reps=1, stages="A1B2C3D", agg_mode="full", num_dev=C, ag_impl="cc",
           f8=True, scratch=None, pipe=2, pres=False, queues=4, spkt=True,
           gbufs=6, pabufs=2, barrier="", wpbufs=3, ppbufs=4, warm=False,
           ptbufs=2):
    KA, KB = meta["KA"], meta["KB"]
    tile_base, lo_base, hi_base = meta["tile_base"], meta["lo_base"], meta["hi_base"]
    NTT, NLOp, NHIp = meta["NTT"], meta["NLOp"], meta["NHIp"]
    noloop = meta.get("noloop", False)
    chunk = meta.get("chunk", CHUNK)
    NIDX = 8 * (NLOp + NHIp)
    HDT = F8 if f8 else BF  # H dtype for layers 1/2 (f8 gather row = 256B)

    nc = bacc.Bacc("TRN2", target_bir_lowering=False, debug=False, num_devices=num_dev,
                   dynamic_dma_scratch_size=scratch if scratch else 16384,
                   num_swdge_queues=queues)

    xs = nc.dram_tensor("xst", [IN, NS], BF, kind="ExternalInput")
    idx16 = nc.dram_tensor("idx16", [P, NIDX], I16, kind="ExternalInput")
    if pres:
        pfulld = nc.dram_tensor("pfull", [P, NTT * P], F8, kind="ExternalInput")
    else:
        locd = nc.dram_tensor("loc", [P, NTT, 1], BF, kind="ExternalInput")
    dinvd = nc.dram_tensor("dinv", [P, NW], F32, kind="ExternalInput")
    w1d = nc.dram_tensor("w1", [IN, HID], BF, kind="ExternalInput")
    w2d = nc.dram_tensor("w2", [HID, HID], BF, kind="ExternalInput")
    w3d = nc.dram_tensor("w3", [HID, OUTP], BF, kind="ExternalInput")
    b1d = nc.dram_tensor("b1r", [P, HID], F32, kind="ExternalInput")
    b2d = nc.dram_tensor("b2r", [P, HID], F32, kind="ExternalInput")
    b3d = nc.dram_tensor("b3r", [P, OUT], F32, kind="ExternalInput")
    iotad = nc.dram_tensor("iota", [P, 1, P], BF, kind="ExternalInput")
    identd = nc.dram_tensor("ident", [P, P], BF, kind="ExternalInput")
    outd = nc.dram_tensor("out", [NS, OUT], F32, kind="ExternalOutput")

    with tile.TileContext(nc) as tc:
        with (
            tc.tile_pool(name="const", bufs=1) as cp,
            tc.tile_pool(name="dram", bufs=1, space="DRAM") as dp,
            tc.tile_pool(name="work", bufs=wpbufs) as wp,
            tc.tile_pool(name="mlo", bufs=gbufs if chunk <= 8 else 2) as mplo,
            tc.tile_pool(name="mhi", bufs=gbufs if chunk <= 8 else 2) as mphi,
            tc.tile_pool(name="pwp", bufs=ppbufs) as pp,
            tc.tile_pool(name="hself", bufs=3) as hp,
            tc.tile_pool(name="accp", bufs=1) as acp,
            tc.tile_pool(name="smax", bufs=3) as sp,
            tc.tile_pool(name="ps_a", bufs=pabufs, space="PSUM") as ps_a,
            tc.tile_pool(name="ps_h", bufs=2, space="PSUM") as ps_h,
            tc.tile_pool(name="ps_t", bufs=ptbufs, space="PSUM") as ps_t,
        ):
            # ---- persistent constants in SBUF
            idx_sb = cp.tile([P, NIDX], I16, name="idx_sb", tag="idx_sb")
            nc.sync.dma_start(out=idx_sb[:], in_=idx16[:])
            if pres:
                pfull_sb = cp.tile([P, NTT * P], F8, name="pfull_sb", tag="pfull_sb")
                nc.sync.dma_start(out=pfull_sb[:], in_=pfulld[:])
            else:
                loc_sb = cp.tile([P, NTT, 1], BF, name="loc_sb", tag="loc_sb")
                nc.sync.dma_start(out=loc_sb[:], in_=locd[:])
                iota_sb = cp.tile([P, 1, P], BF, name="iota_sb", tag="iota_sb")
                nc.sync.dma_start(out=iota_sb[:], in_=iotad[:])
            dinv_sb = cp.tile([P, NW], F32, name="dinv_sb", tag="dinv_sb")
            nc.sync.dma_start(out=dinv_sb[:], in_=dinvd[:])
            ident_sb = cp.tile([P, P], BF, name="ident_sb", tag="ident_sb")
            nc.sync.dma_start(out=ident_sb[:], in_=identd[:])
            ident8_sb = None
            if noloop:
                ident8_sb = cp.tile([P, P], F8, name="ident8_sb", tag="ident8_sb")
                nc.scalar.copy(out=ident8_sb[:], in_=ident_sb[:])

            w_sb = {}
            for nm, dt_, dout in (("w1", w1d, HID), ("w2", w2d, HID), ("w3", w3d, OUTP)):
                t = cp.tile([P, 2 * dout], BF, name=f"{nm}_sb", tag=f"{nm}_sb")
                for kb in range(2):
                    nc.sync.dma_start(
                        out=t[:, kb * dout : (kb + 1) * dout],
                        in_=dt_[kb * P : (kb + 1) * P, :],
                    )
                w_sb[nm] = t
            b_sb = {}
            for nm, dt_, dout in (("b1", b1d, HID), ("b2", b2d, HID), ("b3", b3d, OUT)):
                t = cp.tile([P, dout], F32, name=f"{nm}_sb", tag=f"{nm}_sb")
                nc.sync.dma_start(out=t[:], in_=dt_[:])
                b_sb[nm] = t

            # ---- internal DRAM, split by row-block (lo = windows 0..23,
            # hi = 24..48) so each block AllGathers independently
            h_loc = {}
            h_full = {}
            for r in range(reps):
                for nm0, d, dt_ in (("h1", HID, HDT), ("h2", HID, HDT),
                                    ("h3", OUTP, BF)):
                    nm = f"{nm0}_{r}"
                    h_loc[nm] = (
                        dp.tile([GSPLIT, d], dt_, name=f"{nm}_llo", tag=f"{nm}_llo"),
                        dp.tile([NS - GSPLIT, d], dt_, name=f"{nm}_lhi",
                                tag=f"{nm}_lhi"),
                    )
                    h_full[nm] = (
                        dp.tile([BL0, d], dt_, name=f"{nm}_flo", tag=f"{nm}_flo",
                                addr_space="Shared" if ag_impl == "cc" else "Local"),
                        dp.tile([BL1, d], dt_, name=f"{nm}_fhi", tag=f"{nm}_fhi",
                                addr_space="Shared" if ag_impl == "cc" else "Local"),
                    )

            def h_loc_rows(h_loc_pair, w):
                """(tensor, row0) for window w's rows in the split h_loc."""
                if w < NWLO:
                    return h_loc_pair[0], w * P
                return h_loc_pair[1], (w - NWLO) * P

            def h_from_aT(m, rows, aTs, w_t, dout, h_loc_pair, hdt):
                ph = ps_h.tile([P, dout], F32, name="ph", tag="ph")
                for kb in range(2):
                    nc.tensor.matmul(
                        out=ph[:],
                        lhsT=aTs[kb][:],
                        rhs=w_t[:, kb * dout : (kb + 1) * dout],
                        start=(kb == 0),
                        stop=(kb == 1),
                    )
                h_t = wp.tile([P, dout], hdt, name="h_t", tag="h_t")
                nc.scalar.copy(out=h_t[:], in_=ph[:])
                ht_t, r0 = h_loc_rows(h_loc_pair, m)
                nc.sync.dma_start(out=ht_t[r0 : r0 + rows, :], in_=h_t[:rows, :])

            def h_stage(m, rows, act_ap, w_t, dout, h_loc_t, hdt):
                """act tile [P, 256] bf16 (node-major) -> H tile -> h_loc rows."""
                aTs = []
                for kb in range(2):
                    pt = ps_t.tile([P, P], BF, name=f"pt{kb}", tag=f"pt{kb}")
                    nc.tensor.transpose(
                        out=pt[:],
                        in_=act_ap[:, kb * P : (kb + 1) * P],
                        identity=ident_sb[:],
                    )
                    aT = wp.tile([P, P], BF, name=f"aT{kb}", tag=f"aT{kb}")
                    nc.scalar.copy(out=aT[:], in_=pt[:])
                    aTs.append(aT)
                h_from_aT(m, rows, aTs, w_t, dout, h_loc_t, hdt)

            def allgather(nm, part):
                nc.gpsimd.collective_compute(
                    "AllGather",
                    AX.bypass,
                    replica_groups=[list(range(C))],
                    ins=[h_loc[nm][part][:].opt()],
                    outs=[h_full[nm][part][:].opt()],
                )

            def agg_stage(h_full_pair, elem, dagg, bias_t, w_next, dnext,
                          h_next_pair, last, mdt, hdt_next=BF, h_self_pair=None,
                          nm_next=None):
                """Two-pass aggregation: pass 0 consumes the lo-block gather
                stream (+ self rows) into SBUF accumulators — it only needs
                AG-lo, so it runs while AG-hi is still on the wire; pass 1
                consumes the hi stream, combines, and postprocesses. The next
                layer's AG-lo fires mid-pass-1 (after window 23's conv)."""
                mode = agg_mode
                selfadd = noloop and mode in ("full", "nopost")
                lo_ch = {}
                hi_ch = {}
                issued = {"lo": 0, "hi": 0}

                def issue(stream, cid):
                    pool = mplo if stream == "lo" else mphi
                    base_col = 0 if stream == "lo" else 8 * NLOp
                    t = pool.tile(
                        [P, chunk, elem], mdt, name=f"m{stream}", tag=f"m{stream}"
                    )
                    # each pass drives a single stream, so spread that stream
                    # over ALL queues (odd/even split would idle half of them)
                    qn = cid % queues
                    nc.gpsimd.dma_gather(
                        out_ap=t[:, :, :],
                        in_ap=h_full_pair[0][:, :] if stream == "lo"
                        else h_full_pair[1][:, :],
                        idxs_ap=idx_sb[
                            :, base_col + 8 * chunk * cid : base_col + 8 * chunk * (cid + 1)
                        ],
                        num_idxs=chunk * P,
                        num_idxs_reg=chunk * P,
                        elem_size=elem,
                        queue_num=qn,
                        single_packet=spkt,
                    )
                    (lo_ch if stream == "lo" else hi_ch)[cid] = t

                # ---- pass 0: lo-block tiles + self rows -> SBUF accumulators
                accs = {}
                for w in range(NW):
                    rows = min(P, NS - w * P)
                    tb = int(tile_base[w])
                    ka = int(KA[w])
                    if ka:
                        need = (int(lo_base[w]) + ka - 1) // chunk
                        while issued["lo"] <= need:
                            issue("lo", issued["lo"])
                            issued["lo"] += 1
                    if mode == "gather":
                        continue
                    if ka:
                        Pw = pp.tile([P, ka, P], mdt, name="Pwl", tag="Pwl")
                        nc.vector.tensor_tensor(
                            out=Pw[:],
                            in0=loc_sb[:, tb : tb + ka, :1].to_broadcast([P, ka, P]),
                            in1=iota_sb[:].to_broadcast([P, ka, P]),
                            op=AX.is_equal,
                        )
                    if mode == "ponly":
                        continue
                    nmm = ka + (1 if selfadd else 0)
                    acc = acp.tile([P, dagg], F32, name=f"acc{w}",
                                   tag=f"acc{w}_{dagg}")
                    if nmm == 0:
                        nc.vector.memset(acc[:], 0.0)
                        accs[w] = acc
                        continue
                    pa = ps_a.tile([P, dagg], F32, name="pa", tag="pa")
                    if selfadd:
                        # self-loop contribution: contiguous local rows, no
                        # gather descriptors — identity matmul from h_loc
                        hs = hp.tile([P, dagg], mdt, name="hs", tag="hs")
                        if rows < P:
                            nc.vector.memset(hs[:], 0.0)
                        st_t, r0 = h_loc_rows(h_self_pair, w)
                        nc.sync.dma_start(
                            out=hs[:rows, :],
                            in_=st_t[r0 : r0 + rows, :dagg],
                        )
                        nc.tensor.matmul(
                            out=pa[:],
                            lhsT=ident8_sb[:] if mdt == F8 else ident_sb[:],
                            rhs=hs[:],
                            start=True,
                            stop=(ka == 0),
                        )
                    for kk in range(ka):
                        sid = int(lo_base[w]) + kk
                        t = lo_ch[sid // chunk]
                        nc.tensor.matmul(
                            out=pa[:],
                            lhsT=Pw[:, kk, :],
                            rhs=t[:, sid % chunk, :dagg],
                            start=(kk == 0 and not selfadd),
                            stop=(kk == ka - 1),
                        )
                    nc.scalar.copy(out=acc[:], in_=pa[:])
                    accs[w] = acc

                # ---- pass 1: hi-block tiles + acc -> post (+ next-layer AGs)
                for w in range(NW):
                    rows = min(P, NS - w * P)
                    tb = int(tile_base[w])
                    ka = int(KA[w])
                    kb_ = int(KB[w])
                    if kb_:
                        need = (int(hi_base[w]) + kb_ - 1) // chunk
                        while issued["hi"] <= need:
                            issue("hi", issued["hi"])
                            issued["hi"] += 1
                    if mode == "gather":
                        continue
                    if kb_:
                        Pw = pp.tile([P, kb_, P], mdt, name="Pwh", tag="Pwh")
                        nc.vector.tensor_tensor(
                            out=Pw[:],
                            in0=loc_sb[:, tb + ka : tb + ka + kb_, :1].to_broadcast(
                                [P, kb_, P]
                            ),
                            in1=iota_sb[:].to_broadcast([P, kb_, P]),
                            op=AX.is_equal,
                        )
                    if mode == "ponly":
                        continue
                    if kb_:
                        pa = ps_a.tile([P, dagg], F32, name="pa", tag="pa")
                        for kk in range(kb_):
                            sid = int(hi_base[w]) + kk
                            t = hi_ch[sid // chunk]
                            nc.tensor.matmul(
                                out=pa[:],
                                lhsT=Pw[:, kk, :],
                                rhs=t[:, sid % chunk, :dagg],
                                start=(kk == 0),
                                stop=(kk == kb_ - 1),
                            )
                    if mode == "nopost":
                        continue
                    if kb_:
                        comb = wp.tile([P, dagg], F32, name="comb", tag="comb")
                        nc.vector.tensor_tensor(
                            out=comb[:], in0=pa[:], in1=accs[w][:], op=AX.add
                        )
                        src_ap = comb[:]
                    else:
                        src_ap = accs[w][:]
                    if not last:
                        t1 = wp.tile([P, dagg], F32, name="t1", tag="t1")
                        nc.vector.scalar_tensor_tensor(
                            out=t1[:],
                            in0=src_ap,
                            scalar=dinv_sb[:, w : w + 1],
                            in1=bias_t[:],
                            op0=AX.mult,
                            op1=AX.add,
                        )
                        act_t = wp.tile([P, dagg], BF, name="act_t", tag="act_t")
                        nc.scalar.activation(
                            out=act_t[:],
                            in_=t1[:],
                            func=AF.Relu,
                            scale=dinv_sb[:, w : w + 1],
                        )
                        h_stage(w, rows, act_t[:], w_next, dnext, h_next_pair,
                                hdt_next)
                        if nm_next is not None:
                            if w == NWLO - 1:
                                allgather(nm_next, 0)
                            elif w == NW - 1:
                                allgather(nm_next, 1)
                        continue
                    t1 = sp.tile([P, OUT], F32, name="t1s", tag="t1s")
                    nc.vector.scalar_tensor_tensor(
                        out=t1[:],
                        in0=src_ap,
                        scalar=dinv_sb[:, w : w + 1],
                        in1=bias_t[:],
                        op0=AX.mult,
                        op1=AX.add,
                    )
                    if True:
                        negm = sp.tile([P, 1], F32, name="negm", tag="negm")
                        nc.vector.tensor_reduce(
                            out=negm[:],
                            in_=t1[:],
                            axis=mybir.AxisListType.X,
                            op=AX.max,
                            negate=True,
                        )
                        ex = sp.tile([P, OUT], F32, name="ex", tag="ex")
                        ssum = sp.tile([P, 1], F32, name="ssum", tag="ssum")
                        nc.scalar.activation(
                            out=ex[:],
                            in_=t1[:],
                            func=AF.Exp,
                            bias=negm[:],
                            accum_out=ssum[:],
                        )
                        rinv = sp.tile([P, 1], F32, name="rinv", tag="rinv")
                        nc.vector.reciprocal(out=rinv[:], in_=ssum[:])
                        o = sp.tile([P, OUT], F32, name="o", tag="o")
                        nc.vector.tensor_scalar_mul(out=o[:], in0=ex[:], scalar1=rinv[:])
                        nc.sync.dma_start(
                            out=outd[w * P : w * P + rows, :], in_=o[:rows, :]
                        )

            def stage_A(r):
                h1 = f"h1_{r}"
                for m in range(NW):
                    rows = min(P, NS - m * P)
                    aTs = []
                    for kb in range(2):
                        aT = wp.tile([P, P], BF, name=f"aT{kb}", tag=f"aT{kb}")
                        if rows < P:
                            nc.vector.memset(aT[:], 0.0)
                        nc.sync.dma_start(
                            out=aT[:, :rows],
                            in_=xs[kb * P : (kb + 1) * P, m * P : m * P + rows],
                        )
                        aTs.append(aT)
                    h_from_aT(m, rows, aTs, w_sb["w1"], HID, h_loc[h1], HDT)
                    if "1" in stages:
                        if m == NWLO - 1:
                            allgather(h1, 0)
                        elif m == NW - 1:
                            allgather(h1, 1)

            def stage_B(r):
                agg_stage(
                    h_full[f"h1_{r}"], HID, HID, b_sb["b1"], w_sb["w2"], HID,
                    h_loc[f"h2_{r}"], last=False, mdt=HDT, hdt_next=HDT,
                    h_self_pair=h_loc[f"h1_{r}"],
                    nm_next=f"h2_{r}" if "2" in stages else None,
                )

            def stage_C(r):
                agg_stage(
                    h_full[f"h2_{r}"], HID, HID, b_sb["b2"], w_sb["w3"], OUTP,
                    h_loc[f"h3_{r}"], last=False, mdt=HDT, hdt_next=BF,
                    h_self_pair=h_loc[f"h2_{r}"],
                    nm_next=f"h3_{r}" if "3" in stages else None,
                )

            def stage_D(r):
                agg_stage(
                    h_full[f"h3_{r}"], OUTP, OUT, b_sb["b3"], None, 0, None,
                    last=True, mdt=BF, h_self_pair=h_loc[f"h3_{r}"],
                )

            def swdge_warm():
                # one tiny gather per queue: absorbs first-use SWDGE ring
                # latency under stage A instead of stage B's stream head
                wrmi = cp.tile([P, 8], I16, name="wrmi", tag="wrmi")
                nc.vector.memset(wrmi[:], 0)
                for q in range(queues):
                    wt = wp.tile([P, 1, P], I16, name="wrm_t", tag="wrm_t")
                    nc.gpsimd.dma_gather(
                        out_ap=wt[:, :, :],
                        in_ap=idx16[:, 0:P],
                        idxs_ap=wrmi[:, 0:8],
                        num_idxs=P,
                        num_idxs_reg=P,
                        elem_size=P,
                        elem_step=NIDX,
                        queue_num=q,
                        single_packet=spkt,
                    )

            # AllGathers are embedded in the producing stage (after windows 23
            # and 48); the "1"/"2"/"3" stage letters only gate whether they
            # are emitted.
            emit = {
                "A": stage_A,
                "1": lambda r: None,
                "B": stage_B,
                "2": lambda r: None,
                "C": stage_C,
                "3": lambda r: None,
                "D": stage_D,
            }
            # pair-interleaved emission: within a pair of reps, emit each
            # stage for both reps before moving on, so rep r's collective
            # overlaps rep r^1's compute in every engine's program order.
            for base in range(0, reps, pipe):
                pair = [base + j for j in range(pipe) if base + j < reps]
                if warm:
                    swdge_warm()
                for st in stages:
                    for r in pair:
                        emit[st](r)
                    if "s" in barrier:
                        tc.strict_bb_all_engine_barrier()
                if "r" in barrier:
                    tc.strict_bb_all_engine_barrier()

    nc.compile()
    return nc


# --------------------------------------------------------------------------
# persistent-staging runner (inputs stay device-resident between calls)
# --------------------------------------------------------------------------
def _make_runner(nc, in_maps):
    import jax
    from jax.experimental.shard_map import shard_map
    from jax.sharding import Mesh, NamedSharding, PartitionSpec

    from concourse import bass2jax, mybir as mb

    bass2jax.install_neuronx_cc_hook()

    in_names, out_names, out_avals, zero_shapes = [], [], [], []
    for alloc in nc.m.functions[0].allocations:
        if not isinstance(alloc, mb.MemoryLocationSet):
            continue
        name = alloc.memorylocations[0].name
        if alloc.kind == "ExternalInput":
            in_names.append(name)
        elif alloc.kind == "ExternalOutput":
            out_names.append(name)
            shape = tuple(alloc.tensor_shape)
            dtype = mb.dt.np(alloc.dtype)
            out_avals.append(jax.core.ShapedArray(shape, dtype))
            zero_shapes.append((shape, dtype))
    part_name = nc.partition_id_tensor.name if nc.partition_id_tensor else None
    if part_name is not None and part_name in in_names:
        in_names.remove(part_name)
    n_params = len(in_names)
    n_outs = len(out_names)
    all_names = in_names + out_names + ([part_name] if part_name else [])

    def _body(*args):
        operands = list(args)
        if part_name is not None:
            operands.append(bass2jax.partition_id_tensor())
        outs = bass2jax._bass_exec_p.bind(
            *operands,
            out_avals=tuple(out_avals),
            in_names=tuple(all_names),
            out_names=tuple(out_names),
            lowering_input_output_aliases=(),
            sim_require_finite=True,
            sim_require_nnan=True,
            nc=nc,
        )
        return tuple(outs)

    devices = jax.devices()[:C]
    mesh = Mesh(np.asarray(devices), ("core",))
    in_specs = (PartitionSpec("core"),) * (n_params + n_outs)
    out_specs = (PartitionSpec("core"),) * n_outs
    donate = tuple(range(n_params, n_params + n_outs))
    sharded = jax.jit(
        shard_map(_body, mesh=mesh, in_specs=in_specs, out_specs=out_specs,
                  check_rep=False),
        donate_argnums=donate,
        keep_unused=True,
    )
    sh = NamedSharding(mesh, PartitionSpec("core"))
    in_dev = [
        jax.device_put(
            np.concatenate([np.asarray(in_maps[c][n]) for c in range(C)], axis=0), sh
        )
        for n in in_names
    ]
    import jax.numpy as jnp

    zeros_fn = jax.jit(
        lambda: tuple(
            jnp.zeros((C * s[0], *s[1:]), d) for s, d in zero_shapes
        ),
        out_shardings=tuple(sh for _ in zero_shapes),
    )

    def run(fetch=True):
        zeros = zeros_fn()
        outs = sharded(*in_dev, *zeros)
        jax.block_until_ready(outs)
        if not fetch:
            return None
        return [
            {
                n: np.asarray(outs[i]).reshape(C, *out_avals[i].shape)[c]
                for i, n in enumerate(out_names)
            }
            for c in range(C)
        ]

    return run


# --------------------------------------------------------------------------
# entry points
# --------------------------------------------------------------------------
def _execute(inputs, reps=1, runs=1):
    in_maps, meta = _host_prep(inputs)
    t0 = time.time()
    nc = _build(meta, reps=reps, scratch=65536, pipe=2, queues=4, gbufs=12)
    t1 = time.time()
    walls = []
    res = None
    for _ in range(runs):
        ts = time.time()
        res = bass_utils.run_bass_kernel_spmd(nc, in_maps, list(range(C)))
        walls.append(time.time() - ts)
    out_new = np.concatenate([res.results[c]["out"] for c in range(C)], axis=0)
    out = np.empty_like(out_new)
    out[meta["perm"]] = out_new
    return out, dict(build_s=t1 - t0, walls=walls)


def kernel(**inputs) -> np.ndarray:
    out, _ = _execute(inputs, reps=1, runs=1)
    return out.astype(np.float32)


if __name__ == "__main__":
    rng = np.random.default_rng(0)
    d = {
        "x": rng.standard_normal((N, IN)).astype(np.float32),
        "edge_index": rng.integers(0, N, size=(2, E)).astype(np.int32),
    }
    for i, (di, do) in enumerate(((IN, HID), (HID, HID), (HID, OUT)), 1):
        d[f"W{i}"] = (rng.standard_normal((di, do)) * 0.05).astype(np.float32)
        d[f"b{i}"] = np.zeros(do, np.float32)
        if i < 3:
            d[f"g{i}"] = np.ones(do, np.float32)
            d[f"beta{i}"] = np.zeros(do, np.float32)
            d[f"m{i}"] = (rng.standard_normal(do) * 0.1).astype(np.float32)
            d[f"v{i}"] = rng.uniform(0.5, 1.5, do).astype(np.float32)
    out, info = _execute(d)
    print("out shape:", out.shape, "info:", info)

